# revision 12
# baseline (speedup 1.0000x reference)
"""Trainium2 Bass kernel for nn_ActorCritic (LSTM with done-resets + policy/value heads).

Sharding: batch B=256 split across 8 NeuronCores (32 envs/core). The T=512 scan
runs locally per core; weights are replicated. Host only slices inputs along B
and re-interleaves the per-core [T*32, 13] outputs into [T*256, 13].

Per-core dataflow (everything "transposed": H=128 on partitions, batch on free):
  - x is cast fp32->bf16 via SWDGE DMA (DRAM->DRAM), then transposed to
    [IN, T*32] tiles via DMA-xbar transpose (bf16).
  - xW = W_ih @ x^T + bias precomputed per 64-step chunk into SBUF (bf16),
    laid out [128, t, gate_slot, b] so one [128,128] slice per step holds all
    4 gate chunks in (i,f,o,g) order.
  - Per step: 4 matmuls W_hh^T-chunk @ h_masked (bf16) into one PSUM bank +
    1 identity-matmul accumulating the xW slice; Sigmoid on [i,f,o] and Tanh
    on g read PSUM directly (ScalarE); c/h updates on VectorE (fp32);
    h written straight into the bf16 history buffer hs_all.
  - done-masks are (1-done) broadcast across partitions via a K=1 ones-matmul.
  - Heads: per 128 rows, matmul with hs slice as stationary and W_cat^T
    (13 cols) as moving; bias added on VectorE; DMA'd out contiguously.
  - xW precompute / mask broadcasts / head matmuls are emitted interleaved
    between recurrence steps ("fillers") so the in-order PE queue fills the
    dependency stalls of the serial LSTM chain.
"""

import sys
from contextlib import ExitStack

import numpy as np

sys.path.insert(0, "/opt/trn_rl_repo")

# Problem constants (hardcoded per harness contract).
T = 512
B_FULL = 256
NCORES = 8
BC = B_FULL // NCORES  # 32 envs per core
IN = 292
H = 128
A = 12
NOUT = A + 1  # 13

TCH = 32  # steps per chunk
NCH = T // TCH
TBC = TCH * BC  # 2048 tb-columns per chunk

# K-tiles over IN for the xW matmul: (offset, size). Chosen so every tile's
# x^T data starts at partition 0 of one of the three xbar-transposed tiles
# (col blocks 0:128, 128:256, 164:292 -> k-splits 0:128, 128:164, 164:292).
KSPLITS = [(0, 128), (128, 36), (164, 128)]
XPOSE_COLS = [0, 128, 164]
# gate j (PyTorch order i,f,g,o) -> slot in the [i,f,o,g] psum/xW layout
SLOT = {0: 0, 1: 1, 2: 3, 3: 2}


def build_nc(t_total=T, tch=TCH):
    import concourse.bass as bass
    import concourse.tile as tile
    from concourse import bacc, masks, mybir

    f32 = mybir.dt.float32
    bf16 = mybir.dt.bfloat16
    i32 = mybir.dt.int32
    AF = mybir.ActivationFunctionType
    OP = mybir.AluOpType

    nch = t_total // tch
    tbc = tch * BC

    nc = bacc.Bacc("TRN2", target_bir_lowering=False, debug=False)

    # ---- I/O ----
    x_d = nc.dram_tensor("x", [t_total, BC, IN], f32, kind="ExternalInput").ap()
    done_d = nc.dram_tensor("done", [t_total, BC], i32, kind="ExternalInput").ap()
    h0_d = nc.dram_tensor("h0", [BC, H], f32, kind="ExternalInput").ap()
    c0_d = nc.dram_tensor("c0", [BC, H], f32, kind="ExternalInput").ap()
    wih_d = nc.dram_tensor("W_ih", [4 * H, IN], f32, kind="ExternalInput").ap()
    whh_d = nc.dram_tensor("W_hh", [4 * H, H], f32, kind="ExternalInput").ap()
    bih_d = nc.dram_tensor("b_ih", [1, 4 * H], f32, kind="ExternalInput").ap()
    bhh_d = nc.dram_tensor("b_hh", [1, 4 * H], f32, kind="ExternalInput").ap()
    wpi_d = nc.dram_tensor("W_pi", [A, H], f32, kind="ExternalInput").ap()
    bpi_d = nc.dram_tensor("b_pi", [1, A], f32, kind="ExternalInput").ap()
    wv_d = nc.dram_tensor("W_v", [1, H], f32, kind="ExternalInput").ap()
    bv_d = nc.dram_tensor("b_v", [1, 1], f32, kind="ExternalInput").ap()
    out_d = nc.dram_tensor("out", [t_total * BC, NOUT], f32, kind="ExternalOutput").ap()
    xbf_d = nc.dram_tensor("x_bf16", [t_total * BC, IN], bf16).ap()  # scratch

    with tile.TileContext(nc) as tc, ExitStack() as ctx:
        cst = ctx.enter_context(tc.tile_pool(name="cst", bufs=1))
        big = ctx.enter_context(tc.tile_pool(name="big", bufs=1))
        xwp = ctx.enter_context(tc.tile_pool(name="xwp", bufs=2))
        xtp = ctx.enter_context(tc.tile_pool(name="xtp", bufs=2))
        wk = ctx.enter_context(tc.tile_pool(name="wk", bufs=3))
        ld = ctx.enter_context(tc.tile_pool(name="ld", bufs=2))
        pg_pool = ctx.enter_context(tc.tile_pool(name="pg", bufs=4, space="PSUM"))
        ppre = ctx.enter_context(tc.tile_pool(name="ppre", bufs=2, space="PSUM"))
        pmisc = ctx.enter_context(tc.tile_pool(name="pmisc", bufs=2, space="PSUM"))

        # ---- constants / persistent tiles ----
        ident_f = cst.tile([128, 128], f32, tag="idf", name="idf")
        ident_b = cst.tile([128, 128], bf16, tag="idb", name="idb")
        masks.make_identity(nc, ident_f[:, :])
        masks.make_identity(nc, ident_b[:, :])
        ones_f = cst.tile([1, 128], f32, tag="ones", name="ones")
        nc.gpsimd.memset(ones_f[:, :], 1.0)
        ones_b = cst.tile([1, 128], bf16, tag="onesb", name="onesb")
        nc.gpsimd.memset(ones_b[:, :], 1.0)

        wih_t = [cst.tile([128, 512], bf16, tag=f"wihT{k}", name=f"wihT{k}") for k in range(3)]
        whh_t = [cst.tile([128, 128], bf16, tag=f"whhT{j}", name=f"whhT{j}") for j in range(4)]
        wcat_t = cst.tile([128, 16], bf16, tag="wcatT", name="wcatT")
        bias_cat = cst.tile([128, 4], f32, tag="bias_cat", name="bias_cat")
        bias_bc = cst.tile([128, 16], f32, tag="bias_bc", name="bias_bc")
        m_all = big.tile([128, t_total * BC], f32, tag="m_all", name="m_all")
        hs_all = big.tile([128, (t_total + 1) * BC], bf16, tag="hs_all", name="hs_all")
        m_row = big.tile([1, t_total * BC], bf16, tag="m_row", name="m_row")

        # ---- weight prep ----
        # bias_cat[p, j] = (b_ih + b_hh)[j*128 + p]
        b1 = ld.tile([128, 4], f32, tag="b1", name="b1")
        b2 = ld.tile([128, 4], f32, tag="b2", name="b2")
        nc.sync.dma_start(out=b1[:, :], in_=bih_d.rearrange("a (j p) -> p (a j)", j=4, p=128))
        nc.sync.dma_start(out=b2[:, :], in_=bhh_d.rearrange("a (j p) -> p (a j)", j=4, p=128))
        nc.vector.tensor_add(bias_cat[:, :], b1[:, :], b2[:, :])

        # W_ih^T k-tiles (bf16) via PE transpose of fp32 chunks
        for j in range(4):
            wt = ld.tile([128, IN], f32, tag="wload", name="wload")
            nc.sync.dma_start(out=wt[:, :], in_=wih_d[j * 128:(j + 1) * 128, :])
            for k, (off, sz) in enumerate(KSPLITS):
                pt = pmisc.tile([128, 512], f32, tag="pmisc", name="pmisc")
                nc.tensor.transpose(pt[0:sz, 0:128], wt[:, off:off + sz], ident_f[:, :])
                nc.vector.tensor_copy(wih_t[k][0:sz, j * 128:(j + 1) * 128], pt[0:sz, 0:128])
        # W_hh^T chunks
        for j in range(4):
            wt = ld.tile([128, H], f32, tag="whload", name="whload")
            nc.sync.dma_start(out=wt[:, :], in_=whh_d[j * 128:(j + 1) * 128, :])
            pt = pmisc.tile([128, 512], f32, tag="pmisc", name="pmisc")
            nc.tensor.transpose(pt[0:128, 0:128], wt[:, :], ident_f[:, :])
            nc.vector.tensor_copy(whh_t[j][:, :], pt[0:128, 0:128])
        # W_cat^T = [W_pi; W_v]^T  [128, 13]
        wc = ld.tile([16, H], f32, tag="wcat", name="wcat")
        nc.sync.dma_start(out=wc[0:A, :], in_=wpi_d[:, :])
        nc.sync.dma_start(out=wc[A:A + 1, :], in_=wv_d[:, :])
        pt = pmisc.tile([128, 512], f32, tag="pmisc", name="pmisc")
        nc.tensor.transpose(pt[0:128, 0:NOUT], wc[0:NOUT, :], ident_f[0:NOUT, 0:NOUT])
        nc.vector.tensor_copy(wcat_t[:, 0:NOUT], pt[0:128, 0:NOUT])
        # head bias broadcast [128, 13]
        br = ld.tile([1, 16], f32, tag="brow", name="brow")
        nc.sync.dma_start(out=br[0:1, 0:A], in_=bpi_d[:, :])
        nc.sync.dma_start(out=br[0:1, A:A + 1], in_=bv_d[:, :])
        pt = pmisc.tile([128, 512], f32, tag="pmisc", name="pmisc")
        nc.tensor.matmul(pt[0:128, 0:NOUT], ones_f[0:1, :], br[0:1, 0:NOUT], start=True, stop=True)
        nc.vector.tensor_copy(bias_bc[:, 0:NOUT], pt[0:128, 0:NOUT])

        # ---- masks: m = 1 - done, flattened to one row then PE-broadcast ----
        p_rows = min(128, t_total)
        n_mrow = t_total * BC // p_rows
        done_sb = ld.tile([p_rows, n_mrow], i32, tag="done_sb", name="done_sb")
        m_conv = ld.tile([p_rows, n_mrow], bf16, tag="m_conv", name="m_conv")
        nc.sync.dma_start(out=done_sb[:, :],
                          in_=done_d.rearrange("(p q) b -> p (q b)", p=p_rows))
        nc.vector.tensor_scalar(m_conv[:, :], done_sb[:, :], -1.0, 1.0, OP.mult, OP.add)
        nc.sync.dma_start(out=m_row[0:1, :], in_=m_conv[:, :])

        n_mpieces = t_total * BC // 512

        def emit_mpiece(p):
            pm = pmisc.tile([128, 512], f32, tag="pmisc", name="pmisc")
            nc.tensor.matmul(pm[:, :], ones_b[0:1, :], m_row[0:1, p * 512:(p + 1) * 512],
                             start=True, stop=True)
            nc.scalar.copy(m_all[:, p * 512:(p + 1) * 512], pm[:, :])

        mp_per_ch = (tbc + 511) // 512  # mask pieces per chunk

        # ---- h0/c0 ----
        h0s = ld.tile([BC, H], f32, tag="h0s", name="h0s")
        c0s = ld.tile([BC, H], f32, tag="c0s", name="c0s")
        nc.sync.dma_start(out=h0s[:, :], in_=h0_d[:, :])
        nc.sync.dma_start(out=c0s[:, :], in_=c0_d[:, :])

        # masks for chunk 0 and 1 must exist before h_m0 and the loop
        for p in range(min(2 * mp_per_ch, n_mpieces)):
            emit_mpiece(p)

        pt = pmisc.tile([128, 512], f32, tag="pmisc", name="pmisc")
        nc.tensor.transpose(pt[0:128, 0:BC], h0s[:, :], ident_f[0:BC, 0:BC])
        nc.scalar.copy(hs_all[:, 0:BC], pt[0:128, 0:BC])
        hm_prev = wk.tile([128, BC], bf16, tag="hm", name="hm")
        nc.vector.tensor_mul(hm_prev[:, :], pt[0:128, 0:BC], m_all[:, 0:BC])
        pt = pmisc.tile([128, 512], f32, tag="pmisc", name="pmisc")
        nc.tensor.transpose(pt[0:128, 0:BC], c0s[:, :], ident_f[0:BC, 0:BC])
        ctld_prev = wk.tile([128, BC], f32, tag="ctld", name="ctld")
        nc.vector.tensor_mul(ctld_prev[:, :], pt[0:128, 0:BC], m_all[:, 0:BC])

        # ---- x cast + transpose pipeline ----
        def emit_cast(k):
            if k >= nch:
                return
            nc.gpsimd.dma_start(
                out=xbf_d[k * tbc:(k + 1) * tbc, :],
                in_=x_d[k * tch:(k + 1) * tch, :, :])

        xt_tiles = {}

        def emit_xpose(k):
            if k >= nch:
                return
            tiles = []
            for i, cb in enumerate(XPOSE_COLS):
                xt = xtp.tile([128, tbc], bf16, tag=f"xt{i}", name=f"xt{i}")
                nc.sync.dma_start_transpose(
                    xt[:, :], xbf_d[k * tbc:(k + 1) * tbc, cb:cb + 128])
                tiles.append(xt)
            xt_tiles[k] = tiles

        def make_pre_fillers(k):
            """xW precompute for chunk k: 16 groups of (3 MM + 1 ACT copy)."""
            if k >= nch:
                return []
            xw = xwp.tile([128, tch * 128], bf16, tag="xw", name="xw")
            xw_tiles[k] = xw
            xwv = xw[:, :].rearrange("p (t s b) -> p t s b", t=tch, s=4, b=BC)
            xts = xt_tiles[k]
            fillers = []
            for pc in range(tbc // 512):
                for j in range(4):
                    def f(pc=pc, j=j, xwv=xwv, xts=xts):
                        pp = ppre.tile([128, 512], f32, tag="ppre", name="ppre")
                        first = True
                        for kt, (off, sz) in enumerate(KSPLITS):
                            nc.tensor.matmul(
                                pp[:, :],
                                wih_t[kt][0:sz, j * 128:(j + 1) * 128],
                                xts[kt][0:sz, pc * 512:(pc + 1) * 512],
                                start=first, stop=(kt == 2))
                            first = False
                        s = SLOT[j]
                        nc.scalar.activation(
                            xwv[:, pc * 16:(pc + 1) * 16, s:s + 1, :], pp[:, :],
                            AF.Identity, bias=bias_cat[:, j:j + 1])
                    fillers.append(f)
            return fillers

        def make_head_fillers(k):
            """Head matmuls for chunk k: one group per 128 output rows."""
            if k < 0 or k >= nch:
                return []
            fillers = []
            for c in range(k * (tbc // 128), (k + 1) * (tbc // 128)):
                def f(c=c):
                    ph = pmisc.tile([128, 512], f32, tag="pmisc", name="pmisc")
                    col0 = (4 * c + 1) * BC
                    nc.tensor.matmul(ph[0:128, 0:NOUT],
                                     hs_all[:, col0:col0 + 128],
                                     wcat_t[:, 0:NOUT], start=True, stop=True)
                    ob = wk.tile([128, 16], f32, tag="outsb", name="outsb")
                    nc.vector.tensor_add(ob[:, 0:NOUT], ph[0:128, 0:NOUT],
                                         bias_bc[:, 0:NOUT])
                    nc.sync.dma_start(out=out_d[c * 128:(c + 1) * 128, :],
                                      in_=ob[:, 0:NOUT])
                fillers.append(f)
            return fillers

        def make_mask_fillers(k):
            if k >= nch:
                return []
            lo = k * mp_per_ch
            hi = min((k + 1) * mp_per_ch, n_mpieces)
            return [lambda p=p: emit_mpiece(p) for p in range(lo, hi)]

        xw_tiles = {}
        # prologue: casts for chunks 0..2, transposes 0..1, full precompute chunk 0
        for k in range(min(3, nch)):
            emit_cast(k)
        emit_xpose(0)
        if nch > 1:
            emit_xpose(1)
        for f in make_pre_fillers(0):
            f()

        # ---- the recurrence ----
        for k in range(nch):
            emit_cast(k + 3)
            emit_xpose(k + 2)
            fillers = (make_pre_fillers(k + 1) + make_head_fillers(k - 1)
                       + make_mask_fillers(k + 2))
            xw = xw_tiles[k]
            for tl in range(tch):
                t = k * tch + tl
                pg = pg_pool.tile([128, 128], f32, tag="pg", name="pg")
                nc.tensor.matmul(pg[:, 0:32], whh_t[0][:, :], hm_prev[:, :],
                                 start=True, stop=False)
                nc.tensor.matmul(pg[:, 32:64], whh_t[1][:, :], hm_prev[:, :],
                                 start=False, stop=False)
                nc.tensor.matmul(pg[:, 64:96], whh_t[3][:, :], hm_prev[:, :],
                                 start=False, stop=False)
                nc.tensor.matmul(pg[:, 96:128], whh_t[2][:, :], hm_prev[:, :],
                                 start=False, stop=False)
                nc.tensor.matmul(pg[:, :], ident_b[:, :],
                                 xw[:, tl * 128:(tl + 1) * 128],
                                 start=False, stop=True)

                sig = wk.tile([128, 96], f32, tag="sig", name="sig")
                tg = wk.tile([128, 32], f32, tag="tg", name="tg")
                nc.scalar.activation(sig[:, :], pg[:, 0:96], AF.Sigmoid)
                nc.scalar.activation(tg[:, :], pg[:, 96:128], AF.Tanh)

                t2 = wk.tile([128, 32], f32, tag="t2", name="t2")
                t1 = wk.tile([128, 32], f32, tag="t1", name="t1")
                cn = wk.tile([128, 32], f32, tag="cn", name="cn")
                nc.vector.tensor_mul(t2[:, :], sig[:, 32:64], ctld_prev[:, :])
                nc.vector.tensor_mul(t1[:, :], sig[:, 0:32], tg[:, :])
                nc.vector.tensor_add(cn[:, :], t1[:, :], t2[:, :])
                last = (t == t_total - 1)
                if not last:
                    som = wk.tile([128, 32], f32, tag="som", name="som")
                    nc.vector.tensor_mul(som[:, :], sig[:, 64:96],
                                         m_all[:, (t + 1) * BC:(t + 2) * BC])
                thc = wk.tile([128, 32], f32, tag="thc", name="thc")
                nc.scalar.activation(thc[:, :], cn[:, :], AF.Tanh)
                if not last:
                    hm = wk.tile([128, BC], bf16, tag="hm", name="hm")
                    nc.vector.tensor_mul(hm[:, :], som[:, :], thc[:, :])
                nc.vector.tensor_mul(hs_all[:, (t + 1) * BC:(t + 2) * BC],
                                     sig[:, 64:96], thc[:, :])
                if not last:
                    ctld = wk.tile([128, 32], f32, tag="ctld", name="ctld")
                    nc.vector.tensor_mul(ctld[:, :], cn[:, :],
                                         m_all[:, (t + 1) * BC:(t + 2) * BC])
                    hm_prev = hm
                    ctld_prev = ctld

                if fillers:
                    fillers.pop(0)()
            for f in fillers:
                f()
        for f in make_head_fillers(nch - 1):
            f()

    nc.compile()
    return nc


_NC = None


def _get_nc():
    global _NC
    if _NC is None:
        _NC = build_nc()
    return _NC


def _make_in_maps(inputs):
    x = np.asarray(inputs["x"], dtype=np.float32)
    done = np.asarray(inputs["done"], dtype=np.int32)
    h0 = np.asarray(inputs["h0"], dtype=np.float32)
    c0 = np.asarray(inputs["c0"], dtype=np.float32)
    shared = {
        "W_ih": np.ascontiguousarray(inputs["W_ih"], dtype=np.float32),
        "W_hh": np.ascontiguousarray(inputs["W_hh"], dtype=np.float32),
        "b_ih": np.asarray(inputs["b_ih"], dtype=np.float32).reshape(1, 4 * H),
        "b_hh": np.asarray(inputs["b_hh"], dtype=np.float32).reshape(1, 4 * H),
        "W_pi": np.ascontiguousarray(inputs["W_pi"], dtype=np.float32),
        "b_pi": np.asarray(inputs["b_pi"], dtype=np.float32).reshape(1, A),
        "W_v": np.ascontiguousarray(inputs["W_v"], dtype=np.float32),
        "b_v": np.asarray(inputs["b_v"], dtype=np.float32).reshape(1, 1),
    }
    in_maps = []
    for c in range(NCORES):
        sl = slice(c * BC, (c + 1) * BC)
        in_maps.append({
            "x": np.ascontiguousarray(x[:, sl, :]),
            "done": np.ascontiguousarray(done[:, sl]),
            "h0": np.ascontiguousarray(h0.reshape(B_FULL, H)[sl]),
            "c0": np.ascontiguousarray(c0.reshape(B_FULL, H)[sl]),
            **shared,
        })
    return in_maps


def _try_device_reset():
    try:
        import ctypes

        import jax

        jax.devices()
        lib = ctypes.CDLL("/opt/axon/libaxon_pjrt.so")
        if hasattr(lib, "axon_reset"):
            lib.axon_reset.restype = ctypes.c_int64
            lib.axon_reset()
    except Exception:
        pass


def kernel(**inputs):
    from concourse.bass_utils import run_bass_kernel_spmd

    nc = _get_nc()
    in_maps = _make_in_maps(inputs)
    try:
        res = run_bass_kernel_spmd(nc, in_maps, core_ids=list(range(NCORES)))
    except Exception:
        _try_device_reset()
        res = run_bass_kernel_spmd(nc, in_maps, core_ids=list(range(NCORES)))
    outs = [r["out"].reshape(T, BC, NOUT) for r in res.results]
    return np.stack(outs, axis=1).reshape(T * B_FULL, NOUT).copy()
